# revision 23
# baseline (speedup 1.0000x reference)
"""Trainium2 Bass kernel for an 8-expert top-2 MoE layer (nn_MoE_8383776161864).

Strategy: data-parallel over tokens across 8 NeuronCores (no collectives).
Each core processes T/8 = 512 tokens and runs the dense gate-masked MoE:
    logits = x @ wg.T ; top-2 softmax gates (zero elsewhere)
    out    = sum_e gate[:, e] * (gelu(x @ wfc[e].T) @ wproj[e].T)
Matmuls run in float32r (FP22 read-reduced fp32) at full PE speed.

All shapes are hardcoded; inputs are the full (unsharded) tensors:
    hidden_states [2, 2048, 1024] f32
    w_gate  [8, 1024] f32
    w_fc    [8, 512, 1024] f32
    w_proj  [8, 1024, 512] f32
Returns [2, 2048, 1024] f32.
"""

import os
import sys

import numpy as np

E = 8
H = 1024
I = 512
B, S = 2, 2048
T = B * S
NCORES = 8
TS = T // NCORES  # 512 tokens per core
KT = H // 128  # 8 k-tiles over H
IT = I // 128  # 4 tiles over I

_cache = {}


def _import_concourse():
    try:
        import concourse  # noqa: F401
    except ImportError:
        for p in ("/opt/trn_rl_repo", "/root/.axon_site/_ro/trn_rl_repo"):
            if os.path.isdir(p) and p not in sys.path:
                sys.path.insert(0, p)
        import concourse  # noqa: F401


def build_nc():
    """Build the per-core Bass module (identical program on all 8 cores)."""
    _import_concourse()
    import concourse.tile as tile
    from concourse import bacc, mybir
    from concourse.masks import make_identity

    f32 = mybir.dt.float32
    f32r = mybir.dt.float32r
    bf16 = mybir.dt.bfloat16

    nc = bacc.Bacc(None, target_bir_lowering=False, debug=False)

    xT = nc.dram_tensor("xT", [H, TS], f32r, kind="ExternalInput")  # x^T shard
    wgT = nc.dram_tensor("wgT", [H, E], f32, kind="ExternalInput")  # w_gate^T
    wfcT = nc.dram_tensor("wfcT", [E, H, I], f32r, kind="ExternalInput")  # per-e w_fc^T
    wprT = nc.dram_tensor("wprT", [E, I, H], f32r, kind="ExternalInput")  # per-e w_proj^T
    outT = nc.dram_tensor("outT", [H, TS], f32, kind="ExternalOutput")

    with tile.TileContext(nc) as tc:
        with (
            tc.tile_pool(name="xp", bufs=1) as xp,
            tc.tile_pool(name="consts", bufs=1) as consts,
            tc.tile_pool(name="gatesp", bufs=1) as gatesp,
            tc.tile_pool(name="wfcp", bufs=3) as wfcp,
            tc.tile_pool(name="wprp", bufs=3) as wprp,
            tc.tile_pool(name="hmidp", bufs=2) as hmidp,
            tc.tile_pool(name="yaccp", bufs=1) as yaccp,
            tc.tile_pool(name="ph", bufs=3, space="PSUM") as ph,
            tc.tile_pool(name="py", bufs=3, space="PSUM") as py,
            tc.tile_pool(name="routp", bufs=4) as routp,
        ):
            # ---- load x^T as 8 k-tiles [128, TS] ----
            x_sb = xp.tile([128, KT, TS], f32r)
            for k in range(KT):
                for half in range(2):
                    hs = slice(half * (TS // 2), (half + 1) * (TS // 2))
                    eng = nc.sync if (2 * k + half) % 2 == 0 else nc.gpsimd
                    eng.dma_start(
                        x_sb[:, k, hs], xT[k * 128 : (k + 1) * 128, hs]
                    )

            # ---- constants ----
            ident = consts.tile([128, 128], f32)
            make_identity(nc, ident)
            # eye8[k, m + 128*e] = 1 iff k == e : one-hot row blocks for the
            # partition-broadcast matmul (out[m, t] = gatesT[e, t]).
            eye8_np = np.zeros((8, E * 128), np.float32)
            for e in range(E):
                eye8_np[e, e * 128 : (e + 1) * 128] = 1.0
            eye8_dram = nc.inline_tensor(eye8_np, name="eye8c")
            eye8 = consts.tile([8, E * 128], f32r)
            nc.sync.dma_start(eye8, eye8_dram[:, :].bitcast(f32r))

            wg_sb = consts.tile([128, KT, E], f32)
            for k in range(KT):
                nc.sync.dma_start(wg_sb[:, k, :], wgT[k * 128 : (k + 1) * 128, :])

            def load_fc(e):
                wfc_sb = wfcp.tile([128, KT, I], f32r)
                for k in range(KT):
                    nc.sync.dma_start(
                        wfc_sb[:, k, :], wfcT[e, k * 128 : (k + 1) * 128, :]
                    )
                return wfc_sb

            def load_pr(e):
                wpr_sb = wprp.tile([128, IT, H], f32r)
                for kk in range(IT):
                    for half in range(2):
                        hs = slice(half * 512, (half + 1) * 512)
                        nc.gpsimd.dma_start(
                            wpr_sb[:, kk, hs], wprT[e, kk * 128 : (kk + 1) * 128, hs]
                        )
                return wpr_sb

            # ---- router: logits, top-2 softmax gates (token-major) ----
            # gbc[:, e, :] broadcasts gate column e across all 128 partitions.
            gbc = gatesp.tile([128, E, TS], f32)
            gatesT_sb = gatesp.tile([8, TS], f32r)
            if True:
                for tt in range(TS // 128):
                    ts_ = slice(tt * 128, (tt + 1) * 128)
                    plog = py.tile([128, E], f32, tag="pm2")
                    for k in range(KT):
                        nc.tensor.matmul(
                            plog,
                            x_sb[:, k, ts_].bitcast(f32),
                            wg_sb[:, k, :],
                            start=(k == 0),
                            stop=(k == KT - 1),
                        )
                    logit = routp.tile([128, E], f32)
                    nc.vector.tensor_copy(logit, plog)
                    mx = routp.tile([128, 8], f32)
                    nc.vector.max(mx, logit)  # top-8 sorted desc
                    m1 = mx[:, 0:1]
                    m2 = mx[:, 1:2]
                    sm = routp.tile([128, 4], f32)
                    d21, e21, den, w1 = (sm[:, i : i + 1] for i in range(4))
                    nc.vector.tensor_sub(d21, m2, m1)
                    nc.scalar.activation(e21, d21, mybir.ActivationFunctionType.Exp)
                    nc.vector.tensor_scalar_add(den, e21, 1.0)
                    nc.vector.reciprocal(w1, den)  # w1 = 1/(1+exp(m2-m1))
                    w2 = routp.tile([128, 1], f32)
                    nc.vector.tensor_mul(w2, e21, w1)  # w2 = 1 - w1
                    g1 = routp.tile([128, E], f32)
                    g2 = routp.tile([128, E], f32)
                    nc.vector.tensor_scalar(
                        g1, logit, m1, w1,
                        op0=mybir.AluOpType.is_equal, op1=mybir.AluOpType.mult,
                    )
                    nc.vector.tensor_scalar(
                        g2, logit, m2, w2,
                        op0=mybir.AluOpType.is_equal, op1=mybir.AluOpType.mult,
                    )
                    gates = routp.tile([128, E], f32)
                    nc.vector.tensor_add(gates, g1, g2)
                    # transpose [128, E] -> [E, 128] via PE
                    ptr = py.tile([8, 128], f32, tag="pm2")
                    nc.tensor.transpose(ptr[:E, :], gates, ident)
                    nc.vector.tensor_copy(gatesT_sb[:E, ts_], ptr[:E, :])
                # broadcast each expert's gate row across 128 partitions
                for e in range(E):
                    pbc = ph.tile([128, TS], f32, tag="pm1")
                    nc.tensor.matmul(
                        pbc,
                        eye8[:, e * 128 : (e + 1) * 128],
                        gatesT_sb,
                        start=True,
                        stop=True,
                    )
                    nc.scalar.activation(
                        gbc[:, e, :], pbc, mybir.ActivationFunctionType.Copy
                    )

            # ---- expert loop (software pipelined) ----
            fc0 = load_fc(0)
            y_acc = yaccp.tile([128, KT, TS], f32)

            def mm1(e, wfc_sb):
                hmid = hmidp.tile([128, IT, TS], f32r)
                for m in range(IT):
                    pm = ph.tile([128, TS], f32, tag="pm1")
                    for k in range(KT):
                        nc.tensor.matmul(
                            pm,
                            wfc_sb[:, k, m * 128 : (m + 1) * 128],
                            x_sb[:, k, :],
                            start=(k == 0),
                            stop=(k == KT - 1),
                        )
                    nc.scalar.activation(
                        hmid[:, m, :], pm, mybir.ActivationFunctionType.Gelu
                    )
                    nc.vector.tensor_mul(hmid[:, m, :], hmid[:, m, :], gbc[:, e, :])
                return hmid

            def mm2(e, wpr_sb, hmid):
                for m in range(KT):
                    pm = py.tile([128, TS], f32, tag="pm2")
                    for kk in range(IT):
                        nc.tensor.matmul(
                            pm,
                            wpr_sb[:, kk, m * 128 : (m + 1) * 128],
                            hmid[:, kk, :],
                            start=(kk == 0),
                            stop=(kk == IT - 1),
                        )
                    if e == 0:
                        nc.scalar.activation(
                            y_acc[:, m, :], pm, mybir.ActivationFunctionType.Copy
                        )
                    else:
                        nc.vector.tensor_add(y_acc[:, m, :], y_acc[:, m, :], pm)

            fcs = {1: load_fc(1)}
            prs = {0: load_pr(0)}
            hmids = {}
            for e in range(E):
                if e + 2 < E:
                    fcs[e + 2] = load_fc(e + 2)
                hmids[e] = mm1(e, fcs.pop(e) if e else fc0)
                if e + 1 < E:
                    prs[e + 1] = load_pr(e + 1)
                if e >= 1:
                    mm2(e - 1, prs.pop(e - 1), hmids.pop(e - 1))
            mm2(E - 1, prs.pop(E - 1), hmids.pop(E - 1))

            for m in range(KT):
                for q in range(4):
                    hs = slice(q * (TS // 4), (q + 1) * (TS // 4))
                    eng = nc.sync if q % 2 == 0 else nc.gpsimd
                    eng.dma_start(
                        outT[m * 128 : (m + 1) * 128, hs], y_acc[:, m, hs]
                    )

    nc.compile()
    return nc


def _prep_inputs(hidden_states, w_gate, w_fc, w_proj):
    x = np.ascontiguousarray(
        np.asarray(hidden_states, dtype=np.float32).reshape(T, H).T
    )  # [H, T]
    wgT = np.ascontiguousarray(np.asarray(w_gate, dtype=np.float32).T)  # [H, E]
    wfcT = np.ascontiguousarray(
        np.transpose(np.asarray(w_fc, dtype=np.float32), (0, 2, 1))
    )  # [E, H, I]
    wprT = np.ascontiguousarray(
        np.transpose(np.asarray(w_proj, dtype=np.float32), (0, 2, 1))
    )  # [E, I, H]
    in_maps = [
        {
            "xT": np.ascontiguousarray(x[:, c * TS : (c + 1) * TS]),
            "wgT": wgT,
            "wfcT": wfcT,
            "wprT": wprT,
        }
        for c in range(NCORES)
    ]
    return in_maps


def run(in_maps, trace=False):
    _import_concourse()
    from concourse.bass_utils import run_bass_kernel_spmd

    if "nc" not in _cache:
        _cache["nc"] = build_nc()
    nc = _cache["nc"]
    return run_bass_kernel_spmd(
        nc, in_maps, core_ids=list(range(NCORES)), trace=trace
    )

def kernel(hidden_states, w_gate, w_fc, w_proj):
    in_maps = _prep_inputs(hidden_states, w_gate, w_fc, w_proj)
    res = run(in_maps, trace=False)
    outs = [res.results[c]["outT"] for c in range(NCORES)]
    full = np.concatenate(outs, axis=1)  # [H, T]
    return np.ascontiguousarray(full.T).reshape(B, S, H).astype(np.float32)


# revision 24
# speedup vs baseline: 1.0882x; 1.0882x over previous
"""Trainium2 Bass kernel for an 8-expert top-2 MoE layer (nn_MoE_8383776161864).

Strategy: data-parallel over tokens across 8 NeuronCores (no collectives).
Each core processes T/8 = 512 tokens and runs the dense gate-masked MoE:
    logits = x @ wg.T ; top-2 softmax gates (zero elsewhere)
    out    = sum_e gate[:, e] * (gelu(x @ wfc[e].T) @ wproj[e].T)

The expert matmuls run in float32r (FP22 read-reduced fp32, full PE speed at
free-dim >= 256, ~1e-4 relative error); the tiny router matmul runs in exact
fp32 so top-2 selection matches the fp32 reference (FP22 logits flip near-tie
tokens). Gates are built without data-dependent control flow: vector.max gives
the top-2 values, equality masks select the experts, exp/reciprocal form the
2-way softmax, a PE transpose + one-hot matmul broadcasts each expert's gate
row across partitions. Expert weights stream from HBM double-buffered; the
second matmul accumulates over experts into an SBUF tile via DVE adds.

All shapes are hardcoded; kernel() takes the full (unsharded) inputs:
    hidden_states [2, 2048, 1024] f32
    w_gate  [8, 1024] f32
    w_fc    [8, 512, 1024] f32
    w_proj  [8, 1024, 512] f32
and returns the full [2, 2048, 1024] f32 output.
"""

import os
import sys

import numpy as np

E = 8
H = 1024
I = 512
B, S = 2, 2048
T = B * S
NCORES = 8
TS = T // NCORES  # 512 tokens per core
KT = H // 128  # 8 k-tiles over H
IT = I // 128  # 4 tiles over I

_cache = {}


def _import_concourse():
    try:
        import concourse  # noqa: F401
    except ImportError:
        for p in ("/opt/trn_rl_repo", "/root/.axon_site/_ro/trn_rl_repo"):
            if os.path.isdir(p) and p not in sys.path:
                sys.path.insert(0, p)
        import concourse  # noqa: F401


def build_nc():
    """Build the per-core Bass module (identical program on all 8 cores)."""
    _import_concourse()
    import concourse.tile as tile
    from concourse import bacc, mybir
    from concourse.masks import make_identity

    f32 = mybir.dt.float32
    f32r = mybir.dt.float32r

    nc = bacc.Bacc(None, target_bir_lowering=False, debug=False)

    xT = nc.dram_tensor("xT", [H, TS], f32r, kind="ExternalInput")  # x^T shard
    wgT = nc.dram_tensor("wgT", [H, E], f32, kind="ExternalInput")  # w_gate^T
    wfcT = nc.dram_tensor("wfcT", [E, H, I], f32r, kind="ExternalInput")
    wprT = nc.dram_tensor("wprT", [E, I, H], f32r, kind="ExternalInput")
    outT = nc.dram_tensor("outT", [H, TS], f32, kind="ExternalOutput")

    with tile.TileContext(nc) as tc:
        with (
            tc.tile_pool(name="xp", bufs=1) as xp,
            tc.tile_pool(name="consts", bufs=1) as consts,
            tc.tile_pool(name="gatesp", bufs=1) as gatesp,
            tc.tile_pool(name="wfcp", bufs=2) as wfcp,
            tc.tile_pool(name="wprp", bufs=3) as wprp,
            tc.tile_pool(name="hmidp", bufs=2) as hmidp,
            tc.tile_pool(name="yaccp", bufs=1) as yaccp,
            tc.tile_pool(name="ph", bufs=3, space="PSUM") as ph,
            tc.tile_pool(name="py", bufs=3, space="PSUM") as py,
            tc.tile_pool(name="routp", bufs=4) as routp,
        ):
            # ---- load x^T as 8 k-tiles [128, TS] ----
            x_sb = xp.tile([128, KT, TS], f32r)
            for k in range(KT):
                nc.sync.dma_start(x_sb[:, k, :], xT[k * 128 : (k + 1) * 128, :])

            # ---- constants ----
            ident = consts.tile([128, 128], f32)
            make_identity(nc, ident)
            # eye8[k, m + 128*e] = 1 iff k == e : one-hot row blocks for the
            # partition-broadcast matmul (out[m, t] = gatesT[e, t]).
            eye8_np = np.zeros((8, E * 128), np.float32)
            for e in range(E):
                eye8_np[e, e * 128 : (e + 1) * 128] = 1.0
            eye8_dram = nc.inline_tensor(eye8_np, name="eye8c")
            eye8 = consts.tile([8, E * 128], f32r)
            nc.sync.dma_start(eye8, eye8_dram[:, :].bitcast(f32r))

            wg_sb = consts.tile([128, KT, E], f32)
            for k in range(KT):
                nc.sync.dma_start(wg_sb[:, k, :], wgT[k * 128 : (k + 1) * 128, :])

            # ---- router: logits, top-2 softmax gates (token-major) ----
            # gbc[:, e, :] broadcasts gate column e across all 128 partitions.
            gbc = gatesp.tile([128, E, TS], f32)
            gatesT_sb = gatesp.tile([8, TS], f32r)
            for tt in range(TS // 128):
                ts_ = slice(tt * 128, (tt + 1) * 128)
                plog = py.tile([128, E], f32, tag="pm2")
                for k in range(KT):
                    nc.tensor.matmul(
                        plog,
                        x_sb[:, k, ts_].bitcast(f32),
                        wg_sb[:, k, :],
                        start=(k == 0),
                        stop=(k == KT - 1),
                    )
                logit = routp.tile([128, E], f32)
                nc.vector.tensor_copy(logit, plog)
                mx = routp.tile([128, 8], f32)
                nc.vector.max(mx, logit)  # top-8 values, descending
                m1 = mx[:, 0:1]
                m2 = mx[:, 1:2]
                sm = routp.tile([128, 4], f32)
                d21, e21, den, w1 = (sm[:, i : i + 1] for i in range(4))
                nc.vector.tensor_sub(d21, m2, m1)
                nc.scalar.activation(e21, d21, mybir.ActivationFunctionType.Exp)
                nc.vector.tensor_scalar_add(den, e21, 1.0)
                nc.vector.reciprocal(w1, den)  # w1 = 1/(1+exp(m2-m1))
                w2 = routp.tile([128, 1], f32)
                nc.vector.tensor_mul(w2, e21, w1)  # w2 = 1 - w1
                g1 = routp.tile([128, E], f32)
                g2 = routp.tile([128, E], f32)
                nc.vector.tensor_scalar(
                    g1, logit, m1, w1,
                    op0=mybir.AluOpType.is_equal, op1=mybir.AluOpType.mult,
                )
                nc.vector.tensor_scalar(
                    g2, logit, m2, w2,
                    op0=mybir.AluOpType.is_equal, op1=mybir.AluOpType.mult,
                )
                gates = routp.tile([128, E], f32)
                nc.vector.tensor_add(gates, g1, g2)
                # transpose [128, E] -> [E, 128] via PE
                ptr = py.tile([8, 128], f32, tag="pm2")
                nc.tensor.transpose(ptr[:E, :], gates, ident)
                nc.vector.tensor_copy(gatesT_sb[:E, ts_], ptr[:E, :])
            # broadcast each expert's gate row across 128 partitions
            for e in range(E):
                pbc = ph.tile([128, TS], f32, tag="pm1")
                nc.tensor.matmul(
                    pbc,
                    eye8[:, e * 128 : (e + 1) * 128],
                    gatesT_sb,
                    start=True,
                    stop=True,
                )
                nc.vector.tensor_copy(gbc[:, e, :], pbc)

            # ---- expert loop (software pipelined) ----
            y_acc = yaccp.tile([128, KT, TS], f32)

            def load_weights(e):
                wfc_sb = wfcp.tile([128, KT, I], f32r)
                for k in range(KT):
                    nc.sync.dma_start(
                        wfc_sb[:, k, :], wfcT[e, k * 128 : (k + 1) * 128, :]
                    )
                wpr_sb = wprp.tile([128, IT, H], f32r)
                for kk in range(IT):
                    for half in range(2):
                        hs = slice(half * 512, (half + 1) * 512)
                        nc.sync.dma_start(
                            wpr_sb[:, kk, hs], wprT[e, kk * 128 : (kk + 1) * 128, hs]
                        )
                return wfc_sb, wpr_sb

            def mm1(e, wfc_sb):
                hmid = hmidp.tile([128, IT, TS], f32r)
                for m in range(IT):
                    pm = ph.tile([128, TS], f32, tag="pm1")
                    for k in range(KT):
                        nc.tensor.matmul(
                            pm,
                            wfc_sb[:, k, m * 128 : (m + 1) * 128],
                            x_sb[:, k, :],
                            start=(k == 0),
                            stop=(k == KT - 1),
                        )
                    nc.scalar.activation(
                        hmid[:, m, :], pm, mybir.ActivationFunctionType.Gelu
                    )
                    nc.vector.tensor_mul(hmid[:, m, :], hmid[:, m, :], gbc[:, e, :])
                return hmid

            def mm2(e, wpr_sb, hmid):
                for m in range(KT):
                    pm = py.tile([128, TS], f32, tag="pm2")
                    for kk in range(IT):
                        nc.tensor.matmul(
                            pm,
                            wpr_sb[:, kk, m * 128 : (m + 1) * 128],
                            hmid[:, kk, :],
                            start=(kk == 0),
                            stop=(kk == IT - 1),
                        )
                    if e == 0:
                        nc.vector.tensor_copy(y_acc[:, m, :], pm)
                    else:
                        nc.vector.tensor_add(y_acc[:, m, :], y_acc[:, m, :], pm)

            weights = {0: load_weights(0)}
            prev = None  # (e, wpr_sb, hmid)
            for e in range(E):
                if e + 1 < E:
                    weights[e + 1] = load_weights(e + 1)
                wfc_sb, wpr_sb = weights.pop(e)
                hmid = mm1(e, wfc_sb)
                if prev is not None:
                    mm2(*prev)
                prev = (e, wpr_sb, hmid)
            mm2(*prev)

            for m in range(KT):
                nc.sync.dma_start(outT[m * 128 : (m + 1) * 128, :], y_acc[:, m, :])

    nc.compile()
    return nc


def _prep_inputs(hidden_states, w_gate, w_fc, w_proj):
    x = np.ascontiguousarray(
        np.asarray(hidden_states, dtype=np.float32).reshape(T, H).T
    )  # [H, T]
    wgT = np.ascontiguousarray(np.asarray(w_gate, dtype=np.float32).T)  # [H, E]
    wfcT = np.ascontiguousarray(
        np.transpose(np.asarray(w_fc, dtype=np.float32), (0, 2, 1))
    )  # [E, H, I]
    wprT = np.ascontiguousarray(
        np.transpose(np.asarray(w_proj, dtype=np.float32), (0, 2, 1))
    )  # [E, I, H]
    in_maps = [
        {
            "xT": np.ascontiguousarray(x[:, c * TS : (c + 1) * TS]),
            "wgT": wgT,
            "wfcT": wfcT,
            "wprT": wprT,
        }
        for c in range(NCORES)
    ]
    return in_maps


def run(in_maps, trace=False):
    _import_concourse()
    from concourse.bass_utils import run_bass_kernel_spmd

    if "nc" not in _cache:
        _cache["nc"] = build_nc()
    nc = _cache["nc"]
    return run_bass_kernel_spmd(
        nc, in_maps, core_ids=list(range(NCORES)), trace=trace
    )


def kernel(hidden_states, w_gate, w_fc, w_proj):
    in_maps = _prep_inputs(hidden_states, w_gate, w_fc, w_proj)
    res = run(in_maps, trace=False)
    outs = [res.results[c]["outT"] for c in range(NCORES)]
    full = np.concatenate(outs, axis=1)  # [H, T]
    return np.ascontiguousarray(full.T).reshape(B, S, H).astype(np.float32)
